# revision 3
# baseline (speedup 1.0000x reference)
"""LSTM autoencoder (4 stacked Keras-style LSTM layers, relu cell activation)
on 8 Trainium2 NeuronCores.

Strategy: SEQUENCE-parallel. Each core gets the FULL batch (B=64, used as the
matmul moving dim so the per-step U-weight loads are amortized over 64 cols
instead of 8) and a 128-step slice of T=1024, preceded by a burn-in prefix:
LSTM state error from a wrong (zero) initial state decays by ~sigmoid(z_f)
per step (~0.5 typical), so after W=32 burn-in steps per layer the state has
converged far below the 2e-2 tolerance. Layer l starts its recurrence W steps
before layer l+1 so every layer gets a fully-converged input for its own
burn-in window. Core 0's prefix is zero-padded x, which with b=0 keeps the
state exactly zero until t=0. No inter-core communication.

Matmuls run in bf16 (weights + h + x) so the compiler's fast-weight-load
engages; PSUM accumulation and the cell state stay fp32. c >= 0 always
(i,f,g >= 0), so h = o*relu(c) = o*c. Gate elementwise work is split across
DVE / ACT / GpSimd. Per-core the 4 layers run as a chunk-lagged wavefront so
the 4 recurrence dependency chains hide each other's latency.
"""

import sys

sys.path.insert(0, "/opt/trn_rl_repo")

import numpy as np
import ml_dtypes

import concourse.bass as bass
import concourse.bacc as bacc
import concourse.mybir as mybir
import concourse.tile as tile
from concourse.bass_utils import run_bass_kernel_spmd

F32 = mybir.dt.float32
BF16 = mybir.dt.bfloat16
NPBF16 = ml_dtypes.bfloat16
AF = mybir.ActivationFunctionType
ALU = mybir.AluOpType

B, T, INPUT_LEN = 64, 1024, 256
NCORES = 8
TSLICE = T // NCORES  # 128 valid output steps per core
TC = 8  # timesteps per chunk
W_CH = 2  # burn-in chunks per layer boundary (W = 16 steps)
NL = 4
NV = TSLICE // TC  # 8 valid chunks
NCH = NV + NL * W_CH  # 16 absolute chunks for layer 0
NPH = NCH + NL - 1  # wavefront phases
NHAL = max(1, TC * B // 512)  # inproj moving-dim splits (cap 512)
HW_I = TC // NHAL
# (in_features, hidden) per layer
LAYERS = [(256, 256), (256, 128), (128, 256), (256, 256)]

_CACHE = {}


def _gate_perm(h):
    # reference gate order in the 4H axis: i, f, g, o  ->  ours: i, f, o, g
    return np.concatenate(
        [np.arange(0, h), np.arange(h, 2 * h), np.arange(3 * h, 4 * h), np.arange(2 * h, 3 * h)]
    )


def _fold_w(w):
    # [K, N] -> [128, (K//128) * N] with K-tiles side by side (partition major)
    k, n = w.shape
    kt = k // 128
    return np.ascontiguousarray(
        w.reshape(kt, 128, n).transpose(1, 0, 2).reshape(128, kt * n)
    ).astype(NPBF16)


def _build():
    nc = bacc.Bacc("TRN2", target_bir_lowering=False, debug=False, num_devices=NCORES)

    xT_d = nc.dram_tensor("xT", [128, 2, NCH * TC, B], BF16, kind="ExternalInput")
    out_d = nc.dram_tensor("outT", [128, 2, TSLICE, B], BF16, kind="ExternalOutput")
    w_d, u_d, b_d = [], [], []
    for li, (f, h) in enumerate(LAYERS):
        kf, kh, m = f // 128, h // 128, 4 * h // 128
        w_d.append(nc.dram_tensor(f"W{li}", [128, kf * 4 * h], BF16, kind="ExternalInput"))
        u_d.append(nc.dram_tensor(f"U{li}", [128, kh * 4 * h], BF16, kind="ExternalInput"))
        b_d.append(nc.dram_tensor(f"b{li}", [128, m], F32, kind="ExternalInput"))

    with tile.TileContext(nc) as tc:
        with (
            tc.tile_pool(name="const", bufs=1) as cpool,
            tc.tile_pool(name="state", bufs=1) as spool,
            tc.tile_pool(name="xin", bufs=2) as xpool,
            tc.tile_pool(name="zpsum", bufs=1, space="PSUM") as zpp,
            tc.tile_pool(name="ipsum", bufs=2, space="PSUM") as ipp,
        ):
            w_sb, u_sb, b_sb = [], [], []
            zx_sb, hist_sb = [], []
            c_st, g_sb, t1_sb, t2_sb, zps = [], [], [], [], []
            for li, (f, h) in enumerate(LAYERS):
                kf, kh, m = f // 128, h // 128, 4 * h // 128
                w_sb.append(cpool.tile([128, kf * 4 * h], BF16, tag=f"w{li}", name=f"w{li}"))
                u_sb.append(cpool.tile([128, kh * 4 * h], BF16, tag=f"u{li}", name=f"u{li}"))
                b_sb.append(cpool.tile([128, m], F32, tag=f"b{li}", name=f"b{li}"))
                nc.sync.dma_start(w_sb[li][:], w_d[li][:])
                nc.sync.dma_start(u_sb[li][:], u_d[li][:])
                nc.sync.dma_start(b_sb[li][:], b_d[li][:])
                zx_sb.append(spool.tile([128, m, TC, B], BF16, tag=f"zx{li}", name=f"zx{li}"))
                # hist: layer li's output h, double-buffered by chunk parity.
                # slot 0 = h from the last step of the previous chunk.
                hist_sb.append(
                    [
                        spool.tile([128, kh, TC + 1, B], BF16, tag=f"hist{li}_{par}", name=f"hist{li}_{par}")
                        for par in range(2)
                    ]
                )
                c_st.append(spool.tile([128, kh, 1, B], F32, tag=f"c{li}", name=f"c{li}"))
                g_sb.append(spool.tile([128, 3 * kh, 1, B], BF16, tag=f"g{li}", name=f"g{li}"))
                t1_sb.append(spool.tile([128, kh, 1, B], BF16, tag=f"t1{li}", name=f"t1{li}"))
                t2_sb.append(spool.tile([128, kh, 1, B], BF16, tag=f"t2{li}", name=f"t2{li}"))
                zps.append(zpp.tile([128, m, 1, B], F32, tag=f"zp{li}", name=f"zp{li}"))

            def step_mm(li, iv, par):
                h = LAYERS[li][1]
                kh, m = h // 128, 4 * h // 128
                fh = 4 * h
                zp = zps[li]
                hb = hist_sb[li][par]
                for mi in range(m):
                    for k in range(kh):
                        nc.tensor.matmul(
                            zp[:, mi, :, :],
                            u_sb[li][:, k * fh + mi * 128 : k * fh + (mi + 1) * 128],
                            hb[:, k, bass.ds(iv, 1), :],
                            start=(k == 0),
                            stop=(k == kh - 1),
                        )

            def step_zadd(li, iv):
                zp = zps[li]
                nc.vector.tensor_add(zp[:], zp[:], zx_sb[li][:, :, bass.ds(iv, 1), :])

            def step_sig(li):
                kh = LAYERS[li][1] // 128
                nc.scalar.activation(g_sb[li][:], zps[li][:, 0 : 3 * kh, :, :], AF.Sigmoid)

            def step_ig(li):
                # t1 = relu(g_raw) * i   (fused on DVE)
                kh = LAYERS[li][1] // 128
                nc.vector.scalar_tensor_tensor(
                    t1_sb[li][:],
                    zps[li][:, 3 * kh : 4 * kh, :, :],
                    0.0,
                    g_sb[li][:, 0:kh, :, :],
                    op0=ALU.max,
                    op1=ALU.mult,
                )

            def step_fc(li):
                kh = LAYERS[li][1] // 128
                nc.gpsimd.tensor_mul(t2_sb[li][:], g_sb[li][:, kh : 2 * kh, :, :], c_st[li][:])

            def step_cadd(li):
                nc.gpsimd.tensor_add(c_st[li][:], t1_sb[li][:], t2_sb[li][:])

            def step_h(li, iv, par):
                # h = o * c  (c >= 0 so relu(c) = c); write bf16 into hist slot iv+1
                kh = LAYERS[li][1] // 128
                eng = nc.vector if li in (0, 2) else nc.gpsimd
                eng.tensor_mul(
                    hist_sb[li][par][:, :, bass.ds(iv + 1, 1), :],
                    g_sb[li][:, 2 * kh : 3 * kh, :, :],
                    c_st[li][:],
                )

            def inproj(li, src):
                # zx_l = W_l^T @ src + b_l for a whole chunk.
                # src[k] must yield [128, TC, B] slices (bf16).
                f, h = LAYERS[li]
                kf, m = f // 128, 4 * h // 128
                fh = 4 * h
                hw = HW_I
                for mi in range(m):
                    for half in range(NHAL):
                        ps = ipp.tile([128, hw, B], F32, tag="ip", name="ip")
                        for k in range(kf):
                            nc.tensor.matmul(
                                ps[:],
                                w_sb[li][:, k * fh + mi * 128 : k * fh + (mi + 1) * 128],
                                src(k, half),
                                start=(k == 0),
                                stop=(k == kf - 1),
                            )
                        nc.scalar.activation(
                            zx_sb[li][:, mi, half * hw : (half + 1) * hw, :],
                            ps[:],
                            AF.Identity,
                            bias=b_sb[li][:, mi : mi + 1],
                        )

            xts = {}

            def fetch_x(a):
                xt = xpool.tile([128, 2, TC, B], BF16, tag="xt", name="xt")
                nc.sync.dma_start(xt[:], xT_d[:, :, a * TC : (a + 1) * TC, :])
                xts[a] = xt

            fetch_x(0)
            for p in range(NPH):
                if p + 1 < NCH:
                    fetch_x(p + 1)
                # layer li processes absolute chunk a = p - li if within range
                active = [li for li in range(NL) if W_CH * li <= p - li < NCH]
                for li in active:
                    a = p - li
                    par = a % 2
                    hb = hist_sb[li][par]
                    kh = LAYERS[li][1] // 128
                    if a == W_CH * li:
                        nc.gpsimd.memset(hb[:, :, 0:1, :], 0.0)
                        nc.gpsimd.memset(c_st[li][:], 0.0)
                    else:
                        nc.gpsimd.tensor_copy(
                            hb[:, :, 0:1, :], hist_sb[li][1 - par][:, :, TC : TC + 1, :]
                        )
                    if li == 0:
                        xt = xts.pop(a)
                        inproj(0, lambda k, half: xt[:, k, half * HW_I : (half + 1) * HW_I, :])
                    else:
                        src_hb = hist_sb[li - 1][par]
                        inproj(
                            li,
                            lambda k, half: src_hb[
                                :, k, 1 + half * HW_I : 1 + (half + 1) * HW_I, :
                            ],
                        )
                # phase-major issue order: each engine's FIFO interleaves the
                # active layers' chain ops so the chains overlap instead of
                # serializing layer-by-layer through the queues. Fully
                # unrolled: a For_i hardware loop would put a cross-engine
                # drain bubble at every back-edge, which re-throttles the PE
                # clock (HAM) and stalls the wavefront.
                for u in range(TC):
                    for li in active:
                        step_mm(li, u, (p - li) % 2)
                    for li in active:
                        step_zadd(li, u)
                    for li in active:
                        step_sig(li)
                    for li in active:
                        step_ig(li)
                    for li in active:
                        step_fc(li)
                    for li in active:
                        step_cadd(li)
                    for li in active:
                        step_h(li, u, (p - li) % 2)
                if NL - 1 in active:
                    a = p - (NL - 1)
                    if a >= NCH - NV:
                        av = a - (NCH - NV)
                        nc.sync.dma_start(
                            out_d[:, :, av * TC : (av + 1) * TC, :],
                            hist_sb[NL - 1][a % 2][:, :, 1 : TC + 1, :],
                        )
    nc.compile()
    return nc


def _prep_inputs(x, ws, us, bs):
    base = {}
    for li, (f, h) in enumerate(LAYERS):
        perm = _gate_perm(h)
        base[f"W{li}"] = _fold_w(ws[li][:, perm])
        base[f"U{li}"] = _fold_w(us[li][:, perm])
        base[f"b{li}"] = np.ascontiguousarray(bs[li][perm].reshape(4 * h // 128, 128).T)
    pad = NCH * TC - TSLICE  # burn-in prefix length (128)
    in_maps = []
    for ci in range(NCORES):
        s = ci * TSLICE
        lo = s - pad
        xc = np.zeros((B, NCH * TC, INPUT_LEN), dtype=np.float32)
        xc[:, max(lo, 0) - lo :, :] = x[:, max(lo, 0) : s + TSLICE, :]
        xT = np.ascontiguousarray(
            xc.reshape(B, NCH * TC, 2, 128).transpose(3, 2, 1, 0)
        ).astype(NPBF16)  # [128, 2, NCH*TC, B]
        m = dict(base)
        m["xT"] = xT
        in_maps.append(m)
    return in_maps


def kernel(x, W1, U1, b1, W2, U2, b2, W3, U3, b3, W4, U4, b4):
    x = np.asarray(x, dtype=np.float32)
    ws = [np.asarray(a, np.float32) for a in (W1, W2, W3, W4)]
    us = [np.asarray(a, np.float32) for a in (U1, U2, U3, U4)]
    bs = [np.asarray(a, np.float32) for a in (b1, b2, b3, b4)]

    if "nc" not in _CACHE:
        _CACHE["nc"] = _build()
    nc = _CACHE["nc"]

    in_maps = _prep_inputs(x, ws, us, bs)
    _CACHE["last_in_maps"] = in_maps

    res = run_bass_kernel_spmd(nc, in_maps, list(range(NCORES)))
    out = np.empty((B, T, INPUT_LEN), dtype=np.float32)
    for ci in range(NCORES):
        oT = np.asarray(res.results[ci]["outT"]).astype(np.float32)  # [128, 2, 128, B]
        out[:, ci * TSLICE : (ci + 1) * TSLICE, :] = oT.transpose(3, 2, 1, 0).reshape(
            B, TSLICE, INPUT_LEN
        )
    return out
